# revision 28
# baseline (speedup 1.0000x reference)
"""Trainium2 Bass kernel for nn_LlamaAttention_31782757990403 (v3).

Sparse (full + streaming) Llama attention block with W8A8 fake-quant
projections, distributed over 8 NeuronCores (uniform SPMD, one NEFF).
Core c owns kv head c (query heads 4c..4c+3) for all 4 batches.

Key optimizations over the original baseline:
  - int8 weight transport (qkv_w, o_w) with on-chip int8->bf16 upconvert
    split across DVE/ACT (DVE-heavy: DVE runs the copy in 2x mode),
    pipelined with the weight DMA. Halves weight HBM traffic.
  - All DMAs have >=512B innermost contiguous runs (host prepacks
    partition-major layouts; V rows pre-padded with the ones column).
  - Streaming-head KV packed position-permuted so slot validity depends
    on the partition only (1088 = 34*32 real slots at p<34 in every
    chunk) -> one [128,1] exp bias for all chunks on every core,
    enabling 8-chunk grouped exp on ACT (4 ACT ops per unit vs 32).
  - Weight scales (ws) folded into host-precomputed rope tables.
  - KV loaded in half-tiles so score matmuls start before V arrives.
  - Attention output gathered in f16 (526KB collective vs 1MB f32),
    carrying the per-core partial-|max| row in the same collective.
  - Requant after the gather: global scale derived twice (token-
    partition via transpose+reduce for the final dequant; feature-
    partition via a broadcast re-read of the gathered pmax rows +
    free-axis reduce), avoiding any DRAM scale bounce.
  - o_w load + upconvert + output-side prep hide in the gather window.

Numerics: identical int8 fake-quant math as the reference; attention in
fp16 with f32 PSUM accumulation and constant -4 exp shift (cancels).
Host-simulated rel err vs reference: 5.4e-3 (tolerance 2e-2).
"""

import numpy as np
import ml_dtypes

import concourse.bass as bass
import concourse.mybir as mybir
import concourse.tile as tile
from concourse import bacc, bass_utils
from concourse.masks import make_identity

dt = mybir.dt
AF = mybir.ActivationFunctionType
ALU = mybir.AluOpType
AX = mybir.AxisListType

NH, NKV, HD, HID = 32, 8, 128, 4096
BSZ, QLEN, PLEN = 4, 16, 4096
TOK = BSZ * QLEN                      # 64
G = NH // NKV                         # 4 query heads per kv head
N_CORES = 8
QKV_ROWS = G * HD + 2 * HD            # 768 rows of qkv_w per core
OW_ROWS = HID // N_CORES              # 512 o_w rows per core
NCH = PLEN // HD                      # 32 past-kv chunks of 128
SCL = float(1.0 / np.sqrt(np.float32(HD)))   # 1/sqrt(128)
SHIFT = -4.0                          # exp stability shift (cancels)
NEG = -1.0e9
VROW = HD + 1                         # v row with ones column
KVW = PLEN + NCH * VROW               # 8224 = packed per-unit kv width
CB = 513                              # contrib rows: 512 attn + 1 pmax
SL_REAL = 34                          # streaming real partitions (34*32=1088)
KHALF = 16 * HD                       # 2048 cols of kT per half
VHALF = 16 * VROW                     # 2064 cols of v per half

_prog_cache = {}


def _build_program():
    nc = bacc.Bacc("TRN2", target_bir_lowering=False, debug=False,
                   num_devices=N_CORES)
    f32, f16, bf16, i32, i8 = (dt.float32, dt.float16, dt.bfloat16,
                               dt.int32, dt.int8)

    def inp(name, shape, d):
        return nc.dram_tensor(name, shape, d, kind="ExternalInput").ap()

    xq8 = inp("xq8", [HD, 32 * TOK], i8)            # [p, kc*64+t]
    wq8 = inp("wq8", [HD, 32 * QKV_ROWS], i8)       # [p, kc*768+f]
    tabs = inp("tabs", [TOK, 4 * 5 * 64], f32)      # cosA|sinA|sinB|cosB
    smalls = inp("smalls", [HD, 198], f32)          # maskb|nmask|xs|wsv|owsT
    kv = inp("kv", [BSZ, HD, KVW], f16)             # kT | v(+ones) packed
    ow8 = inp("ow8", [HD, 32 * OW_ROWS], i8)        # [p, kc*512+r]
    out_ap = nc.dram_tensor("out_slice", [G * HD, TOK], f32,
                            kind="ExternalOutput").ap()

    with tile.TileContext(nc, num_cores=N_CORES) as tc:
        with (
            tc.tile_pool(name="persist", bufs=1) as P1,
            tc.tile_pool(name="w8", bufs=6) as W8P,
            tc.tile_pool(name="wb", bufs=4) as WBP,
            tc.tile_pool(name="kt", bufs=3) as KTP,
            tc.tile_pool(name="vt", bufs=3) as VTP,
            tc.tile_pool(name="pb", bufs=4) as PBP,
            tc.tile_pool(name="ow8", bufs=4) as OW8P,
            tc.tile_pool(name="owb", bufs=4) as OWBP,
            tc.tile_pool(name="wk", bufs=2) as WK,
            tc.tile_pool(name="ps_p", bufs=1, space="PSUM") as PSP,
            tc.tile_pool(name="ps_s", bufs=2, space="PSUM") as PSS,
            tc.tile_pool(name="ps_o", bufs=2, space="PSUM") as PSO,
            tc.tile_pool(name="ps_m", bufs=2, space="PSUM") as PSM,
            tc.tile_pool(name="dram", bufs=1, space="DRAM") as DR,
        ):
            # ---------------- persistent small loads ----------------
            xq8_sb = P1.tile([HD, 32 * TOK], i8)
            nc.sync.dma_start(out=xq8_sb, in_=xq8)
            tabs_sb = P1.tile([TOK, 4 * 5 * 64], f32)
            nc.sync.dma_start(out=tabs_sb, in_=tabs)
            # packed smalls: [maskb | nmask | xs | wsv_b | owsT]
            smalls_sb = P1.tile([HD, 198], f32)
            nc.sync.dma_start(out=smalls_sb, in_=smalls)
            maskb_sb = smalls_sb[:, 0:1]
            nmask_sb = smalls_sb[0:QLEN, 1:65]
            xs_sb = smalls_sb[0:TOK, 65:66]
            wsv_sb = smalls_sb[0:TOK, 66:194]
            owsT_sb = smalls_sb[:, 194:198]

            shift16 = P1.tile([QLEN, 1], f32)
            nc.vector.memset(shift16, SHIFT)
            ones_r = P1.tile([1, HD], f32)
            nc.vector.memset(ones_r, 1.0)
            ident16 = P1.tile([HD, HD], f16)
            make_identity(nc, ident16)
            ident32 = P1.tile([HD, HD], f32)
            make_identity(nc, ident32)
            xqT_sb = P1.tile([HD, 32, TOK], bf16)
            nc.vector.tensor_copy(out=xqT_sb, in_=xq8_sb)

            # ---------------- bulk DMAs (queue order matters) -------
            w8_tiles = []
            for gi in range(8):
                w8 = W8P.tile([HD, 4 * QKV_ROWS], i8, tag="w8")
                nc.sync.dma_start(
                    out=w8, in_=wq8[:, gi * 4 * QKV_ROWS:(gi + 1) * 4 * QKV_ROWS])
                w8_tiles.append(w8)
            kt_tiles, vt_tiles = [], []
            for b in range(BSZ):
                kh = [KTP.tile([HD, KHALF], f16, tag=f"kt{h}",
                               name=f"kt{h}_{b}") for h in range(2)]
                vh = [VTP.tile([HD, VHALF], f16, tag=f"vt{h}",
                               name=f"vt{h}_{b}") for h in range(2)]
                nc.sync.dma_start(out=kh[0], in_=kv[b][:, 0:KHALF])
                nc.sync.dma_start(out=kh[1], in_=kv[b][:, KHALF:PLEN])
                nc.sync.dma_start(out=vh[0], in_=kv[b][:, PLEN:PLEN + VHALF])
                nc.sync.dma_start(out=vh[1], in_=kv[b][:, PLEN + VHALF:KVW])
                kt_tiles.append(kh)
                vt_tiles.append(vh)

            # PE p-state warmup: keep PE busy so proj runs at full clock
            for wi in range(20):
                wps = PSM.tile([HD, HD], f16, tag="misc", name=f"warm{wi}")
                nc.tensor.transpose(wps, ident16, ident16)

            # ---------------- QKV projection (int8 weights) ---------
            # column order within 768: [k(128), q0..q3(512), v(128)]
            ps1 = PSP.tile([TOK, 384], f32, tag="p1", padded_shape=[TOK, 512])
            ps2 = PSP.tile([TOK, 384], f32, tag="p2", padded_shape=[TOK, 512])
            for pi in range(16):                  # chunk pairs
                wb = WBP.tile([HD, 2 * QKV_ROWS], bf16, tag="wb")
                src = w8_tiles[pi // 2][:, (pi % 2) * 2 * QKV_ROWS:
                                        ((pi % 2) + 1) * 2 * QKV_ROWS]
                if pi in (1, 4, 7, 10, 13):
                    nc.scalar.activation(out=wb, in_=src, func=AF.Copy)
                else:
                    nc.vector.tensor_copy(out=wb, in_=src)
                for j in range(2):
                    kc = pi * 2 + j
                    nc.tensor.matmul(ps1, lhsT=xqT_sb[:, kc, :],
                                     rhs=wb[:, j * QKV_ROWS:j * QKV_ROWS + 384],
                                     start=(kc == 0), stop=(kc == 31))
                    nc.tensor.matmul(ps2, lhsT=xqT_sb[:, kc, :],
                                     rhs=wb[:, j * QKV_ROWS + 384:(j + 1) * QKV_ROWS],
                                     start=(kc == 0), stop=(kc == 31))
            qkv_sb = P1.tile([TOK, QKV_ROWS], f32)
            nc.scalar.activation(out=qkv_sb[:, 0:384], in_=ps1, func=AF.Copy,
                                 scale=xs_sb[:, 0:1])
            nc.scalar.activation(out=qkv_sb[:, 384:768], in_=ps2, func=AF.Copy,
                                 scale=xs_sb[:, 0:1])

            # ---------------- RoPE (ws folded into tables) ----------
            # segs 0..4 = [k, q0..q3] at cols 0..639; rotate-half pairs
            # (x1, x2) = (cols j, cols 64+j) within each 128-col segment.
            def seg3(ap2d, off):
                return bass.AP(tensor=ap2d.tensor, offset=ap2d.offset + off,
                               ap=[ap2d.ap[0]] + [[128, 5], [1, 64]])

            def tab3(i):
                sl = tabs_sb[:, i * 320:(i + 1) * 320]
                return bass.AP(tensor=sl.tensor, offset=sl.offset,
                               ap=[sl.ap[0]] + [[64, 5], [1, 64]])

            # v dequant first (feeds the gpsimd-queue v16 bounces early)
            v16f = P1.tile([TOK, HD], f16)
            nc.vector.tensor_mul(out=v16f, in0=qkv_sb[:, 640:768], in1=wsv_sb)
            v16 = P1.tile([QLEN, BSZ, VROW], f16)
            for b in range(BSZ):
                nc.gpsimd.dma_start(out=v16[:, b, 0:HD],
                                    in_=v16f[b * QLEN:(b + 1) * QLEN, :])
            nc.vector.memset(v16[:, :, HD:HD + 1], 1.0)

            roped = P1.tile([TOK, 5 * HD], f32)
            t1 = P1.tile([TOK, 320], f32)
            t2 = P1.tile([TOK, 320], f32)
            x1 = seg3(qkv_sb[:, 0:640], 0)
            x2 = seg3(qkv_sb[:, 0:640], 64)
            nc.vector.tensor_mul(out=t1, in0=x1, in1=tab3(0))   # x1*cosA
            nc.vector.tensor_mul(out=t2, in0=x2, in1=tab3(1))   # x2*sinA
            nc.vector.tensor_sub(out=seg3(roped[:, :], 0), in0=t1, in1=t2)
            nc.vector.tensor_mul(out=t1, in0=x1, in1=tab3(2))   # x1*sinB
            nc.vector.tensor_mul(out=t2, in0=x2, in1=tab3(3))   # x2*cosB
            nc.vector.tensor_add(out=seg3(roped[:, :], 64), in0=t1, in1=t2)

            # transpose k + q heads: [64, 128] f32 -> [128, 64] f16
            qT_sb = P1.tile([HD, 5, TOK], f16)
            for seg in range(5):
                pst = PSM.tile([HD, TOK], f32, tag="misc")
                nc.tensor.transpose(pst, roped[:, seg * HD:(seg + 1) * HD],
                                    ident32[0:TOK, 0:TOK])
                nc.vector.tensor_copy(out=qT_sb[:, seg, :], in_=pst)

            # ---------------- attention units ----------------------
            # software-pipelined across (unit, chunk-group): score matmuls
            # of the next group run on PE while ACT does the current exp,
            # and the next unit's scores precede this unit's late PVs.
            contrib = DR.tile([CB, TOK], f16)
            gathered = DR.tile([N_CORES * CB, TOK], f16, addr_space="Shared")
            pm_all = P1.tile([TOK, BSZ], f32)

            o_tiles = [None] * BSZ
            s_tiles = {}
            p_tiles = {}
            sn_tiles = {}
            pn_tiles = {}

            def qtu(b):
                return qT_sb[:, 1:5, b * QLEN:(b + 1) * QLEN]  # [128,4,16]

            def sc(b, g):
                s_ps = PSS.tile([HD, 512], f32, tag="sc", name=f"sc{b}{g}")
                kt = kt_tiles[b][g // 2]
                for j in range(8):
                    lk = (g % 2) * 8 + j
                    nc.tensor.matmul(
                        s_ps[:, j * TOK:(j + 1) * TOK],
                        lhsT=kt[:, lk * HD:(lk + 1) * HD],
                        rhs=qtu(b), start=True, stop=True)
                s_tiles[(b, g)] = s_ps

            def ex(b, g):
                p_big = PBP.tile([HD, 512], f16, tag="pb", name=f"pb{b}{g}")
                nc.scalar.activation(out=p_big, in_=s_tiles[(b, g)],
                                     func=AF.Exp, scale=SCL,
                                     bias=maskb_sb[:, 0:1])
                p_tiles[(b, g)] = p_big

            def pv(b, g):
                if o_tiles[b] is None:
                    o_tiles[b] = PSO.tile([TOK, VROW], f32, tag="o",
                                          name=f"ops{b}")
                o_ps = o_tiles[b]
                p_big = p_tiles[(b, g)]
                vt = vt_tiles[b][g // 2]
                for j in range(8):
                    lk = (g % 2) * 8 + j
                    nc.tensor.matmul(
                        o_ps, lhsT=p_big[:, j * TOK:(j + 1) * TOK],
                        rhs=vt[:, lk * VROW:(lk + 1) * VROW],
                        start=(g == 0 and j == 0), stop=False)

            def snmm(b):
                s_n = PSM.tile([QLEN, TOK], f32, tag="misc", name=f"sn{b}")
                nc.tensor.matmul(s_n, lhsT=qT_sb[:, 0, b * QLEN:(b + 1) * QLEN],
                                 rhs=qtu(b), start=True, stop=True)
                sn_tiles[b] = s_n

            def snfin(b):
                s_n = sn_tiles[b]
                nc.vector.tensor_add(out=s_n, in0=s_n, in1=nmask_sb)
                p_n = WK.tile([QLEN, TOK], f16, tag="pn", name=f"pn{b}")
                nc.scalar.activation(out=p_n, in_=s_n, func=AF.Exp,
                                     scale=SCL, bias=shift16[:, 0:1])
                pn_tiles[b] = p_n

            def pvnew(b):
                nc.tensor.matmul(o_tiles[b], lhsT=pn_tiles[b],
                                 rhs=v16[:, b, :], start=False, stop=True)

            def epi(b):
                o_ps = o_tiles[b]
                rden = WK.tile([TOK, 1], f32, tag="rden", name=f"rden{b}")
                nc.vector.reciprocal(out=rden, in_=o_ps[:, HD:HD + 1])
                o16 = WK.tile([TOK, HD], f16, tag="o16", name=f"o16_{b}")
                nc.scalar.activation(out=o16, in_=o_ps[:, 0:HD], func=AF.Copy,
                                     scale=rden[:, 0:1])
                nc.vector.tensor_reduce(out=pm_all[:, b:b + 1], in_=o16,
                                        axis=AX.X, op=ALU.max,
                                        apply_absolute_value=True)
                ot_ps = PSM.tile([HD, TOK], f16, tag="misc", name=f"otp{b}")
                nc.tensor.transpose(ot_ps, o16, ident16[0:TOK, 0:TOK])
                ot16 = WK.tile([HD, TOK], f16, tag="ot", bufs=4,
                               name=f"ot{b}")
                nc.vector.tensor_copy(out=ot16, in_=ot_ps)
                # contrib[g*128 + d, b*16 + s] = ot16[d, g*16 + s]
                nc.sync.dma_start(
                    out=bass.AP(tensor=contrib.tensor, offset=b * QLEN,
                                ap=[[TOK, HD], [HD * TOK, G], [1, QLEN]]),
                    in_=ot16.rearrange("p (g s) -> p g s", g=G))

            sc(0, 0); ex(0, 0); sc(0, 1); ex(0, 1); pv(0, 0)
            sc(0, 2); ex(0, 2); pv(0, 1); sc(0, 3); snmm(0); ex(0, 3)
            snfin(0); pv(0, 2); sc(1, 0); ex(1, 0); pv(0, 3); pvnew(0)
            sc(1, 1); ex(1, 1); epi(0); pv(1, 0); sc(1, 2); ex(1, 2)
            pv(1, 1); sc(1, 3); snmm(1); ex(1, 3); snfin(1); pv(1, 2)
            sc(2, 0); ex(2, 0); pv(1, 3); pvnew(1); sc(2, 1); ex(2, 1)
            epi(1); pv(2, 0); sc(2, 2); ex(2, 2); pv(2, 1); sc(2, 3)
            snmm(2); ex(2, 3); snfin(2); pv(2, 2); sc(3, 0); ex(3, 0)
            pv(2, 3); pvnew(2); sc(3, 1); ex(3, 1); epi(2); pv(3, 0)
            sc(3, 2); ex(3, 2); pv(3, 1); sc(3, 3); snmm(3); ex(3, 3)
            snfin(3); pv(3, 2); pv(3, 3); pvnew(3); epi(3)

            # per-core pmax row: [64,4] -> [4,64] -> max over g -> [4,16]
            pm_ps = PSM.tile([BSZ, TOK], f32, tag="misc")
            nc.tensor.transpose(pm_ps, pm_all, ident32[0:TOK, 0:TOK])
            pm_sb = WK.tile([BSZ, TOK], f32, tag="pms")
            nc.vector.tensor_copy(out=pm_sb, in_=pm_ps)
            m1 = WK.tile([BSZ, QLEN], f32, tag="m1")
            nc.vector.tensor_max(out=m1, in0=pm_sb[:, 0:16], in1=pm_sb[:, 16:32])
            m2 = WK.tile([BSZ, QLEN], f32, tag="m2")
            nc.vector.tensor_max(out=m2, in0=pm_sb[:, 32:48], in1=pm_sb[:, 48:64])
            pm16 = WK.tile([BSZ, QLEN], f16, tag="pm16")
            nc.vector.tensor_max(out=pm16, in0=m1, in1=m2)
            nc.scalar.dma_start(
                out=bass.AP(tensor=contrib.tensor, offset=512 * TOK,
                            ap=[[QLEN, BSZ], [1, QLEN]]),
                in_=pm16)

            # ---------------- AllGather (f16, 526KB total) ----------
            nc.gpsimd.collective_compute(
                "AllGather", ALU.bypass,
                replica_groups=[list(range(N_CORES))],
                ins=[contrib.opt()], outs=[gathered.opt()])

            # ---------------- o_w load + upconvert (in gather win) --
            owb_tiles = []
            for gj in range(4):
                o8 = OW8P.tile([HD, 8 * OW_ROWS], i8, tag="o8")
                nc.sync.dma_start(
                    out=o8, in_=ow8[:, gj * 8 * OW_ROWS:(gj + 1) * 8 * OW_ROWS])
                owb = OWBP.tile([HD, 8, OW_ROWS], bf16, tag="owb")
                for q in range(4):
                    sl = slice(q * 2 * OW_ROWS, (q + 1) * 2 * OW_ROWS)
                    if (gj * 4 + q) % 3 == 1:
                        nc.scalar.activation(out=owb[:, 2 * q:2 * q + 2, :],
                                             in_=o8[:, sl], func=AF.Copy)
                    else:
                        nc.vector.tensor_copy(out=owb[:, 2 * q:2 * q + 2, :],
                                              in_=o8[:, sl])
                owb_tiles.append(owb)

            # ---------------- global scales from gathered pmax ------
            # feature-partition broadcast of all cores' pmax rows (first:
            # it gates the requant scale chain)
            pgb = P1.tile([HD, N_CORES, TOK], f16)
            nc.scalar.dma_start(out=pgb, in_=bass.AP(
                tensor=gathered.tensor, offset=512 * TOK,
                ap=[[0, HD], [CB * TOK, N_CORES], [1, TOK]]))
            # token-partition scale (for final dequant)
            pg = P1.tile([N_CORES, TOK], f16)
            nc.scalar.dma_start(out=pg, in_=bass.AP(
                tensor=gathered.tensor, offset=512 * TOK,
                ap=[[CB * TOK, N_CORES], [1, TOK]]))

            pg_ps = PSM.tile([TOK, N_CORES], f16, tag="misc")
            nc.tensor.transpose(pg_ps, pg, ident16[0:N_CORES, 0:N_CORES])
            # PE p-state warmup: queued behind the pg-gated transpose above
            # so it runs in the tail of the gather window, warming PE
            for wi in range(22):
                wps2 = PSM.tile([HD, HD], f16, tag="misc",
                                name=f"warm2_{wi}")
                nc.tensor.transpose(wps2, ident16, ident16)
            s_raw = WK.tile([TOK, 1], f32, tag="sraw")
            nc.vector.tensor_reduce(out=s_raw, in_=pg_ps, axis=AX.X,
                                    op=ALU.max)
            s_at = P1.tile([TOK, 1], f32)
            nc.vector.tensor_scalar(out=s_at, in0=s_raw,
                                    scalar1=float(np.float32(1.0) / np.float32(127.0)),
                                    scalar2=1e-8, op0=ALU.mult, op1=ALU.max)
            # broadcast s_at over the 128 feature partitions via PE
            sat_ps = PSM.tile([1, TOK], f32, tag="misc")
            nc.tensor.transpose(sat_ps, s_at, ident32[0:TOK, 0:TOK])
            sat_row = WK.tile([1, TOK], f32, tag="satr")
            nc.vector.tensor_copy(out=sat_row, in_=sat_ps)
            sat_b = PSM.tile([HD, TOK], f32, tag="misc")
            nc.tensor.matmul(sat_b, lhsT=ones_r, rhs=sat_row,
                             start=True, stop=True)

            pgb_r = bass.AP(tensor=pgb[:, :, :].tensor,
                            offset=pgb[:, :, :].offset,
                            ap=[pgb[:, :, :].ap[0], [1, TOK], [TOK, N_CORES]])
            pmb = WK.tile([HD, TOK], f32, tag="pmb")
            nc.vector.tensor_reduce(out=pmb, in_=pgb_r, axis=AX.X, op=ALU.max)
            r1 = WK.tile([HD, TOK], f32, tag="r1")
            nc.vector.reciprocal(out=r1, in_=pmb)
            rxs_b = P1.tile([HD, TOK], f32)
            nc.vector.tensor_scalar(out=rxs_b, in0=r1, scalar1=127.0,
                                    scalar2=1e8, op0=ALU.mult, op1=ALU.min)
            rb = rxs_b[:, :]
            rxs_b4 = bass.AP(tensor=rb.tensor, offset=rb.offset,
                             ap=[rb.ap[0], [0, 4], [1, TOK]])

            # ---------------- readback attn + requant + o-proj ------
            # split across the SP and ACT dma queues for issue overlap
            a_sb = P1.tile([HD, 32, TOK], f16)
            for c in range(N_CORES):
                eng = nc.sync if c % 2 == 0 else nc.scalar
                eng.dma_start(out=a_sb[:, c * G:(c + 1) * G, :], in_=bass.AP(
                    tensor=gathered.tensor, offset=c * CB * TOK,
                    ap=[[TOK, HD], [HD * TOK, G], [1, TOK]]))

            # transposed o-proj: o_w chunk is the stationary operand, the
            # quantized activations move (64 rows vs 512 per matmul).
            # Each of the 4 output regions accumulates in its OWN psum bank
            # (interleaved sub-bank accumulation regions corrupt each other),
            # borrowing the attention pools' banks which are free by now.
            o_regs = [PSS.tile([HD, TOK], f32, tag="sc", name="opr0"),
                      PSS.tile([HD, TOK], f32, tag="sc", name="opr1"),
                      PSO.tile([HD, TOK], f32, tag="o", name="opr2"),
                      PSO.tile([HD, TOK], f32, tag="o", name="opr3")]
            for g in range(8):
                ti = WK.tile([HD, 4 * TOK], i32, tag="ti")
                nc.vector.tensor_mul(out=ti, in0=a_sb[:, 4 * g:4 * g + 4, :],
                                     in1=rxs_b4)
                qa = WK.tile([HD, 4 * TOK], bf16, tag="qa")
                nc.scalar.activation(out=qa, in_=ti, func=AF.Copy)
                for j in range(4):
                    kc = 4 * g + j
                    for r in range(G):
                        nc.tensor.matmul(
                            o_regs[r],
                            lhsT=owb_tiles[kc // 8][:, kc % 8,
                                           r * HD:(r + 1) * HD],
                            rhs=qa[:, j * TOK:(j + 1) * TOK],
                            start=(kc == 0), stop=(kc == 31))
            o_f = P1.tile([HD, G, TOK], f32)
            o_f2 = P1.tile([HD, G, TOK], f32)
            for r in range(G):
                nc.scalar.activation(out=o_f[:, r, :], in_=o_regs[r],
                                     func=AF.Copy, scale=owsT_sb[:, r:r + 1])
                nc.vector.tensor_mul(out=o_f2[:, r, :], in0=o_f[:, r, :],
                                     in1=sat_b)
            nc.sync.dma_start(out=out_ap, in_=o_f2)

    nc.compile()
    return nc


def _quant_rows(w):
    s = np.maximum(np.max(np.abs(w), axis=1, keepdims=True)
                   / np.float32(127.0), np.float32(1e-8)).astype(np.float32)
    q = np.clip(np.round(w / s), -127.0, 127.0).astype(np.float32)
    return q, s[:, 0]


def kernel(x, past_k, past_v, qkv_w, o_w, q_len, num_full_kv_head,
           sink_size, recent_size):
    q_len = int(q_len); nf = int(num_full_kv_head)
    sink = int(sink_size); recent = int(recent_size)
    assert q_len == QLEN and nf == 4 and sink == 64 and recent == 1024, \
        "kernel compiled for q_len=16, nf=4, sink=64, recent=1024"
    x = np.asarray(x, np.float32)
    past_k = np.asarray(past_k, np.float32)
    past_v = np.asarray(past_v, np.float32)
    qkv_w = np.asarray(qkv_w, np.float32)
    o_w = np.asarray(o_w, np.float32)
    bf16 = ml_dtypes.bfloat16
    f16 = np.float16

    # ---- host prep
    xs = np.maximum(np.max(np.abs(x), axis=1, keepdims=True)
                    / np.float32(127.0), np.float32(1e-8)).astype(np.float32)
    xq = np.clip(np.round(x / xs), -127.0, 127.0).astype(np.float32)
    # xq8[p, kc*64+t] = xq[t, kc*128+p]
    xq8 = np.ascontiguousarray(
        xq.T.reshape(32, HD, TOK).transpose(1, 0, 2).reshape(HD, 32 * TOK)
    ).astype(np.int8)

    wq, ws = _quant_rows(qkv_w)
    owq, ows_all = _quant_rows(o_w)

    # RoPE tables (f32, matching jax reference ops)
    d_half = np.arange(0, HD, 2, dtype=np.float32) / np.float32(HD)
    inv_freq = (np.float32(1.0)
                / np.power(np.float32(10000.0), d_half)).astype(np.float32)
    pos = (PLEN + np.arange(QLEN)).astype(np.float32)
    ang = pos[:, None] * inv_freq[None, :]
    cos16 = np.cos(ang).astype(np.float32)   # [16, 64]
    sin16 = np.sin(ang).astype(np.float32)
    cosT = np.tile(cos16, (BSZ, 1))          # [64, 64]
    sinT = np.tile(sin16, (BSZ, 1))

    # new-token causal mask [new_pos, (g, s)]: allow new_pos <= s
    nm = np.full((QLEN, TOK), NEG, np.float32)
    r = np.arange(QLEN)[:, None]
    s = (np.arange(TOK) % QLEN)[None, :]
    nm[r <= s] = 0.0

    in_maps = []
    for c in range(N_CORES):
        # qkv rows, column order [k, q0..q3, v]
        rows_k = wq[HID + c * HD:HID + (c + 1) * HD]
        rows_q = wq[c * G * HD:(c + 1) * G * HD]
        rows_v = wq[HID + NKV * HD + c * HD:HID + NKV * HD + (c + 1) * HD]
        w_c = np.concatenate([rows_k, rows_q, rows_v], axis=0)  # [768, 4096]
        ws_k = ws[HID + c * HD:HID + (c + 1) * HD]
        ws_q = ws[c * G * HD:(c + 1) * G * HD]
        ws_v = ws[HID + NKV * HD + c * HD:HID + NKV * HD + (c + 1) * HD]
        # wq8[p, kc*768+f] = w_c[f, kc*128+p]
        wq8_c = np.ascontiguousarray(
            w_c.T.reshape(32, HD, QKV_ROWS).transpose(1, 0, 2)
            .reshape(HD, 32 * QKV_ROWS)).astype(np.int8)

        # ws folded rope tables: segs [k, q0..q3]
        wsseg = np.stack([ws_k] + [ws_q[g * HD:(g + 1) * HD] for g in range(G)])
        tab = np.empty((TOK, 4, 5, 64), np.float32)
        for sg in range(5):
            tab[:, 0, sg, :] = cosT * wsseg[sg, :64]    # cosA
            tab[:, 1, sg, :] = sinT * wsseg[sg, 64:]    # sinA
            tab[:, 2, sg, :] = sinT * wsseg[sg, :64]    # sinB
            tab[:, 3, sg, :] = cosT * wsseg[sg, 64:]    # cosB
        tab = np.ascontiguousarray(tab.reshape(TOK, 4 * 5 * 64))

        # packed kv per unit: [128, 8224] = kT[128,4096] | v[128, 32*129]
        kv_c = np.zeros((BSZ, HD, KVW), f16)
        if c < nf:
            for b in range(BSZ):
                K = past_k[b, :, c, :]                  # [4096, 128]
                V = past_v[b, :, c, :]
                kv_c[b, :, :PLEN] = K.T.astype(f16)
                vv = np.zeros((NCH, HD, VROW), f16)
                vv[:, :, :HD] = V.reshape(NCH, HD, HD).astype(f16)
                vv[:, :, HD] = 1.0
                kv_c[b, :, PLEN:] = vv.transpose(1, 0, 2).reshape(HD, NCH * VROW)
            mb = np.full((HD, 1), SHIFT, np.float32)
        else:
            for b in range(BSZ):
                Ke = np.concatenate([past_k[b, :sink, c],
                                     past_k[b, PLEN - recent:, c]], axis=0)
                Ve = np.concatenate([past_v[b, :sink, c],
                                     past_v[b, PLEN - recent:, c]], axis=0)
                # slot (ck, p) <- entry p*32 + ck  for p < 34
                kTr = np.zeros((NCH, HD, HD), f16)      # [ck, p, d]
                kTr[:, :SL_REAL, :] = Ke.reshape(SL_REAL, NCH, HD) \
                    .transpose(1, 0, 2).astype(f16)
                kv_c[b, :, :PLEN] = kTr.transpose(2, 0, 1).reshape(HD, PLEN)
                vv = np.zeros((NCH, HD, VROW), f16)
                vv[:, :SL_REAL, :HD] = Ve.reshape(SL_REAL, NCH, HD) \
                    .transpose(1, 0, 2).astype(f16)
                vv[:, :, HD] = 1.0
                kv_c[b, :, PLEN:] = vv.transpose(1, 0, 2).reshape(HD, NCH * VROW)
            mb = np.where(np.arange(HD)[:, None] < SL_REAL, SHIFT,
                          NEG).astype(np.float32)

        ow_c = owq[c * OW_ROWS:(c + 1) * OW_ROWS]       # [512, 4096]
        ow8_c = np.ascontiguousarray(
            ow_c.T.reshape(32, HD, OW_ROWS).transpose(1, 0, 2)
            .reshape(HD, 32 * OW_ROWS)).astype(np.int8)

        smalls = np.zeros((HD, 198), np.float32)
        smalls[:, 0] = mb[:, 0]
        smalls[:QLEN, 1:65] = nm
        smalls[:TOK, 65] = xs[:, 0]
        smalls[:TOK, 66:194] = ws_v[None, :]
        # owsT[p, r] = ows[c*512 + r*128 + p]
        smalls[:, 194:198] = ows_all[c * OW_ROWS:(c + 1) * OW_ROWS] \
            .reshape(G, HD).T

        in_maps.append({
            "xq8": xq8, "wq8": wq8_c, "tabs": tab, "smalls": smalls,
            "kv": kv_c, "ow8": ow8_c,
        })

    global _last_in_maps
    _last_in_maps = in_maps
    if "nc" not in _prog_cache:
        _prog_cache["nc"] = _build_program()
    nc = _prog_cache["nc"]

    res = bass_utils.run_bass_kernel_spmd(nc, in_maps,
                                          core_ids=list(range(N_CORES)))
    out = np.empty((TOK, HID), np.float32)
    for c in range(N_CORES):
        # device rows are (p, r)-interleaved: row p*4+r holds o_w row r*128+p
        sl = res.results[c]["out_slice"].reshape(HD, G, TOK) \
            .transpose(1, 0, 2).reshape(OW_ROWS, TOK)
        out[:, c * OW_ROWS:(c + 1) * OW_ROWS] = sl.T
    return out
